# revision 4
# baseline (speedup 1.0000x reference)
"""Multi-head attention (b=4, S=2048, E=1024, H=16, dh=64) on 8 TRN2 NeuronCores.

Sharding: core c handles sequence b = c//2 and query-half c%2 (1024 query
tokens), computing all 16 heads for its query slice. K/V projections for the
full sequence are computed on both cores of a pair (duplicated) so that no
cross-core communication is needed at all — the kernel is 8 fully independent
single-core programs.

Per-core dataflow (all matmuls in float32r — fp32 storage, TF32-like matmul
precision at 4x native-fp32 speed):
  1. QKV projection per head-pair p (heads 2p, 2p+1): qkvT tiles
     [feat, tok] = WqkvT-slice.T @ xT, bias fused into the PSUM eviction
     (q weights/bias pre-scaled by 1/sqrt(dh) on host).
  2. PE-transpose of vT into v-natural [kpos, feat] tiles, ones column
     appended (computes the softmax denominator for free in step 4).
  3. Scores transposed: sT[kpos, q] = kT.T @ qT per 128-k tile; ACT exp
     (no max subtraction — logits are N(0,1), well within fp32 range).
  4. attn@V: out_aug[65, q] accumulated over 16 k-tiles; row 64 = softmax
     denominator r[q].
  5. Normalize: recip(r) broadcast across partitions via a K=64 selector
     matmul into PSUM, then one DVE multiply.
  6. Output projection vs full WoutT + bias; DMA yT [1024, 1024q] out.

Host reassembles: out[b, qslice] = yT_c.T. The c%2==1 cores receive xT with
the two sequence halves swapped so the SPMD program can always treat columns
0..1023 as its query slice.
"""

import os
import numpy as np

B, S, E = 4, 2048, 1024
H, DH = 16, 64
SCALE = DH ** (-0.5)
NCORES = 8
NP = H // 2          # head pairs per core (all 16 heads, 8 pairs)
ET = E // 128        # 8 e-tiles
KT = S // 128        # 16 k-tiles
QTOK = S // 2        # 1024 query tokens per core
QC = QTOK // 512     # 2 query chunks
SC = S // 512        # 4 seq chunks

_NC = None
LAST_EXEC_NS = None
LAST_RESULTS = None


def build():
    import concourse.mybir as mybir
    from concourse import bacc
    from concourse.tile import TileContext

    F32 = mybir.dt.float32
    F32R = mybir.dt.float32r
    EXP = mybir.ActivationFunctionType.Exp

    nc = bacc.Bacc(None)
    xT = nc.declare_dram_parameter("xT", [E, S], F32R, isOutput=False)
    wqkvT = nc.declare_dram_parameter("wqkvT", [E, 3 * E], F32R, isOutput=False)
    bqkvP = nc.declare_dram_parameter("bqkvP", [NP, 128, 3], F32, isOutput=False)
    woutT = nc.declare_dram_parameter("woutT", [E, E], F32R, isOutput=False)
    boutP = nc.declare_dram_parameter("boutP", [128, ET], F32, isOutput=False)
    ident2 = nc.declare_dram_parameter("ident2", [128, 64], F32R, isOutput=False)
    onesC = nc.declare_dram_parameter("onesC", [128, 1], F32R, isOutput=False)
    bsel = nc.declare_dram_parameter("bsel", [64, 64], F32, isOutput=False)
    yT = nc.declare_dram_parameter("yT", [E, QTOK], F32, isOutput=True)

    with TileContext(nc) as tc:
        with (
            tc.tile_pool(name="xp", bufs=1) as xp,
            tc.tile_pool(name="cst", bufs=1) as cst,
            tc.tile_pool(name="outu", bufs=1) as outup,
        ):
            xt = []
            for e in range(ET):
                t = xp.tile([128, S], F32R, name=f"xT{e}")
                nc.sync.dma_start(out=t[:], in_=xT[e * 128:(e + 1) * 128, :])
                xt.append(t)
            id2 = cst.tile([128, 64], F32R, name="id2")
            nc.sync.dma_start(out=id2[:], in_=ident2[:])
            ones_t = cst.tile([128, 1], F32R, name="ones")
            nc.sync.dma_start(out=ones_t[:], in_=onesC[:])
            bsel_t = cst.tile([64, 64], F32, name="bsel")
            nc.sync.dma_start(out=bsel_t[:], in_=bsel[:])

            outur = []  # normalized attention output, [feat 128, 1024q] per pair

            with (
                tc.tile_pool(name="wp", bufs=1) as wp,
                tc.tile_pool(name="qkv", bufs=2) as qkvp,
                tc.tile_pool(name="vn", bufs=2) as vnp,
                tc.tile_pool(name="pe", bufs=6) as pep,
                tc.tile_pool(name="stg", bufs=2) as stgp,
                tc.tile_pool(name="rq", bufs=1) as rqp,
                tc.tile_pool(name="mm", bufs=4, space="PSUM") as mmp,
                tc.tile_pool(name="po", bufs=4, space="PSUM") as pop,
            ):
                for p in range(NP):
                    with nc.named_scope(f"pair{p}"):
                        # ---- weights for this head pair
                        w = []
                        for e in range(ET):
                            t = wp.tile([128, 384], F32R, name=f"w{e}")
                            nc.sync.dma_start(
                                out=t[:],
                                in_=wqkvT[e * 128:(e + 1) * 128,
                                          p * 384:(p + 1) * 384],
                            )
                            w.append(t)
                        bt = wp.tile([128, 3], F32, name="bias")
                        nc.sync.dma_start(out=bt[:], in_=bqkvP[p])

                        # ---- QKV projection (q only over query half = cols 0:1024)
                        qT2 = qkvp.tile([128, QTOK], F32R, name="qT2")
                        kT2 = qkvp.tile([128, S], F32R, name="kT2")
                        vT2 = qkvp.tile([128, S], F32R, name="vT2")
                        for ft, (dst, nch) in enumerate(
                            [(qT2, QC), (kT2, SC), (vT2, SC)]
                        ):
                            for ch in range(nch):
                                mm = mmp.tile([128, 512], F32, name="mm")
                                for e in range(ET):
                                    nc.tensor.matmul(
                                        mm[:],
                                        w[e][:, ft * 128:(ft + 1) * 128],
                                        xt[e][:, ch * 512:(ch + 1) * 512],
                                        start=(e == 0), stop=(e == ET - 1),
                                    )
                                nc.vector.tensor_scalar_add(
                                    dst[:, ch * 512:(ch + 1) * 512],
                                    mm[:], bt[:, ft:ft + 1],
                                )

                        # ---- transpose v to natural layout + ones column
                        vns = [[None] * KT for _ in range(2)]
                        for h in range(2):
                            for kt in range(KT):
                                tp = mmp.tile([128, 512], F32R, name="mm")
                                nc.tensor.transpose(
                                    tp[:, 0:64],
                                    vT2[h * 64:(h + 1) * 64,
                                        kt * 128:(kt + 1) * 128],
                                    id2[h * 64:(h + 1) * 64, :],
                                )
                                vn = vnp.tile([128, 65], F32R, name=f"vn{h}_{kt}")
                                nc.vector.tensor_copy(vn[:, 0:64], tp[:, 0:64])
                                nc.vector.tensor_copy(vn[:, 64:65], ones_t[:])
                                vns[h][kt] = vn

                        # ---- attention
                        stage = stgp.tile([128, QTOK], F32, name="stage")
                        rqs = []
                        for h in range(2):
                            rq = rqp.tile([64, QTOK], F32, name=f"rq{h}")
                            nc.vector.memset(rq[:], 0.0)
                            rqs.append(rq)
                        groups = [(h, qc) for h in range(2) for qc in range(QC)]
                        pos = {}
                        for g in groups:
                            pos[g] = pop.tile([128, 512], F32, name="po")
                        for kt in range(KT):
                            pexs = {}
                            for (h, qc) in groups:
                                ps = mmp.tile([128, 512], F32, name="mm")
                                nc.tensor.matmul(
                                    ps[:],
                                    kT2[h * 64:(h + 1) * 64,
                                        kt * 128:(kt + 1) * 128],
                                    qT2[h * 64:(h + 1) * 64,
                                        qc * 512:(qc + 1) * 512],
                                    start=True, stop=True,
                                )
                                pex = pep.tile([128, 512], F32R, name="pe")
                                nc.scalar.activation(pex[:], ps[:], EXP)
                                pexs[(h, qc)] = pex
                            for (h, qc) in groups:
                                nc.tensor.matmul(
                                    pos[(h, qc)][:65, :], vns[h][kt][:],
                                    pexs[(h, qc)][:],
                                    start=(kt == 0), stop=(kt == KT - 1),
                                )
                        for (h, qc) in groups:
                            po = pos[(h, qc)]
                            nc.vector.tensor_copy(
                                stage[h * 64:(h + 1) * 64,
                                      qc * 512:(qc + 1) * 512],
                                po[0:64, :],
                            )
                            nc.vector.reciprocal(
                                rqs[h][0:1, qc * 512:(qc + 1) * 512],
                                po[64:65, :],
                            )

                        # ---- normalize (broadcast recip over 64 partitions via PE)
                        ou = outup.tile([128, QTOK], F32R, name=f"outU{p}")
                        for qc in range(QC):
                            bcc = pop.tile([128, 512], F32, name="po")
                            for h in range(2):
                                nc.tensor.matmul(
                                    bcc[h * 64:(h + 1) * 64, :],
                                    bsel_t[:],
                                    rqs[h][:, qc * 512:(qc + 1) * 512],
                                    start=True, stop=True,
                                )
                            nc.vector.tensor_mul(
                                ou[:, qc * 512:(qc + 1) * 512],
                                stage[:, qc * 512:(qc + 1) * 512],
                                bcc[:],
                            )
                        outur.append(ou)

            # ---- output projection
            with (
                tc.tile_pool(name="wo", bufs=1) as wop,
                tc.tile_pool(name="yp", bufs=3) as yp,
                tc.tile_pool(name="mm2", bufs=4, space="PSUM") as mm2p,
            ):
                wo = []
                for ft in range(ET):
                    t = wop.tile([128, E], F32R, name=f"wo{ft}")
                    nc.sync.dma_start(
                        out=t[:], in_=woutT[ft * 128:(ft + 1) * 128, :]
                    )
                    wo.append(t)
                bo = wop.tile([128, ET], F32, name="bo")
                nc.sync.dma_start(out=bo[:], in_=boutP[:])
                with nc.named_scope("outproj"):
                    for et in range(ET):
                        for qc in range(QC):
                            mm2 = mm2p.tile([128, 512], F32, name="mm2")
                            for ft in range(ET):
                                nc.tensor.matmul(
                                    mm2[:],
                                    wo[ft][:, et * 128:(et + 1) * 128],
                                    outur[ft][:, qc * 512:(qc + 1) * 512],
                                    start=(ft == 0), stop=(ft == ET - 1),
                                )
                            ysb = yp.tile([128, 512], F32, name="ysb")
                            nc.vector.tensor_scalar_add(
                                ysb[:], mm2[:], bo[:, et:et + 1]
                            )
                            nc.sync.dma_start(
                                out=yT[et * 128:(et + 1) * 128,
                                       qc * 512:(qc + 1) * 512],
                                in_=ysb[:],
                            )
    nc.finalize()
    return nc


def _host_inputs(x, Wqkv, bqkv, Wout, bout):
    x = np.asarray(x, dtype=np.float32)
    Wqkv = np.asarray(Wqkv, dtype=np.float32)
    bqkv = np.asarray(bqkv, dtype=np.float32)
    Wout = np.asarray(Wout, dtype=np.float32)
    bout = np.asarray(bout, dtype=np.float32)

    wqkvT = np.empty((E, 3 * E), np.float32)
    bqkvP = np.empty((NP, 128, 3), np.float32)
    for p in range(NP):
        r = slice(p * 128, (p + 1) * 128)
        wqkvT[:, p * 384:p * 384 + 128] = (Wqkv[r] * SCALE).T
        wqkvT[:, p * 384 + 128:p * 384 + 256] = Wqkv[E:][r].T
        wqkvT[:, p * 384 + 256:p * 384 + 384] = Wqkv[2 * E:][r].T
        bqkvP[p, :, 0] = bqkv[r] * SCALE
        bqkvP[p, :, 1] = bqkv[E:][r]
        bqkvP[p, :, 2] = bqkv[2 * E:][r]

    woutT = np.ascontiguousarray(Wout.T)
    boutP = np.ascontiguousarray(bout.reshape(ET, 128).T)
    eye = np.eye(64, dtype=np.float32)
    ident2 = np.concatenate([eye, eye], 0)
    onesC = np.ones((128, 1), np.float32)
    bsel = np.concatenate(
        [np.ones((1, 64), np.float32), np.zeros((63, 64), np.float32)], 0
    )

    in_maps = []
    for c in range(NCORES):
        xb = x[c // 2]
        if c % 2:
            xb = np.concatenate([xb[QTOK:], xb[:QTOK]], 0)
        in_maps.append({
            "xT": np.ascontiguousarray(xb.T),
            "wqkvT": wqkvT,
            "bqkvP": bqkvP,
            "woutT": woutT,
            "boutP": boutP,
            "ident2": ident2,
            "onesC": onesC,
            "bsel": bsel,
        })
    return in_maps


def kernel(x, Wqkv, bqkv, Wout, bout):
    global _NC, LAST_EXEC_NS, LAST_RESULTS
    from concourse.bass_utils import run_bass_kernel_spmd

    if _NC is None:
        _NC = build()
    in_maps = _host_inputs(x, Wqkv, bqkv, Wout, bout)
    trace = bool(os.environ.get("ATTN_TRACE"))
    res = run_bass_kernel_spmd(
        _NC, in_maps, core_ids=list(range(NCORES)), trace=trace
    )
    LAST_EXEC_NS = res.exec_time_ns
    LAST_RESULTS = res

    out = np.empty((B, S, E), np.float32)
    for c in range(NCORES):
        b, half = c // 2, c % 2
        out[b, half * QTOK:(half + 1) * QTOK, :] = res.results[c]["yT"].T
    return out


# revision 5
# speedup vs baseline: 1.1648x; 1.1648x over previous
"""Multi-head attention (b=4, S=2048, E=1024, H=16, dh=64) on 8 TRN2 NeuronCores.

Sharding: core c handles sequence b = c//2 and query-half c%2 (1024 query
tokens), computing all 16 heads for its query slice. K/V projections for the
full sequence are computed on both cores of a pair (duplicated) so that no
cross-core communication is needed at all — the kernel is 8 fully independent
single-core programs.

Per-core dataflow (all matmuls in float32r — fp32 storage, TF32-like matmul
precision at 4x native-fp32 speed):
  1. QKV projection per head-pair p (heads 2p, 2p+1): qkvT tiles
     [feat, tok] = WqkvT-slice.T @ xT, bias fused into the PSUM eviction
     (q weights/bias pre-scaled by 1/sqrt(dh) on host).
  2. PE-transpose of vT into v-natural [kpos, feat] tiles, ones column
     appended (computes the softmax denominator for free in step 4).
  3. Scores transposed: sT[kpos, q] = kT.T @ qT per 128-k tile; ACT exp
     (no max subtraction — logits are N(0,1), well within fp32 range).
  4. attn@V: out_aug[65, q] accumulated over 16 k-tiles; row 64 = softmax
     denominator r[q].
  5. Normalize: recip(r) broadcast across partitions via a K=64 selector
     matmul into PSUM, then one DVE multiply.
  6. Output projection vs full WoutT + bias; DMA yT [1024, 1024q] out.

Host reassembles: out[b, qslice] = yT_c.T. The c%2==1 cores receive xT with
the two sequence halves swapped so the SPMD program can always treat columns
0..1023 as its query slice.
"""

import os
import numpy as np

B, S, E = 4, 2048, 1024
H, DH = 16, 64
SCALE = DH ** (-0.5)
NCORES = 8
NP = H // 2          # head pairs per core (all 16 heads, 8 pairs)
ET = E // 128        # 8 e-tiles
KT = S // 128        # 16 k-tiles
QTOK = S // 2        # 1024 query tokens per core
QC = QTOK // 512     # 2 query chunks
SC = S // 512        # 4 seq chunks

_NC = None
LAST_EXEC_NS = None
LAST_RESULTS = None


def build():
    import concourse.mybir as mybir
    from concourse import bacc
    from concourse.tile import TileContext

    F32 = mybir.dt.float32
    F32R = mybir.dt.float32r
    BF16 = mybir.dt.bfloat16
    EXP = mybir.ActivationFunctionType.Exp

    nc = bacc.Bacc(None)
    xT = nc.declare_dram_parameter("xT", [E, S], F32R, isOutput=False)
    wqkvT = nc.declare_dram_parameter("wqkvT", [E, 3 * E], F32R, isOutput=False)
    bqkvP = nc.declare_dram_parameter("bqkvP", [NP, 128, 3], F32, isOutput=False)
    woutT = nc.declare_dram_parameter("woutT", [E, E], F32R, isOutput=False)
    boutP = nc.declare_dram_parameter("boutP", [128, ET], F32, isOutput=False)
    ident2 = nc.declare_dram_parameter("ident2", [128, 64], F32R, isOutput=False)
    onesC = nc.declare_dram_parameter("onesC", [128, 1], F32R, isOutput=False)
    bsel = nc.declare_dram_parameter("bsel", [64, 64], F32, isOutput=False)
    yT = nc.declare_dram_parameter("yT", [E, QTOK], F32, isOutput=True)

    with TileContext(nc) as tc:
        with (
            tc.tile_pool(name="xp", bufs=1) as xp,
            tc.tile_pool(name="cst", bufs=1) as cst,
            tc.tile_pool(name="outu", bufs=1) as outup,
        ):
            xt = []
            for e in range(ET):
                t = xp.tile([128, S], F32R, name=f"xT{e}")
                nc.sync.dma_start(out=t[:], in_=xT[e * 128:(e + 1) * 128, :])
                xt.append(t)
            id2 = cst.tile([128, 64], F32R, name="id2")
            nc.sync.dma_start(out=id2[:], in_=ident2[:])
            ones_t = cst.tile([128, 1], F32R, name="ones")
            nc.sync.dma_start(out=ones_t[:], in_=onesC[:])
            ones_t16 = cst.tile([128, 1], mybir.dt.bfloat16, name="ones16")
            nc.vector.tensor_copy(ones_t16[:], ones_t[:])
            bsel_t = cst.tile([64, 64], F32, name="bsel")
            nc.sync.dma_start(out=bsel_t[:], in_=bsel[:])

            outur = []  # normalized attention output, [feat 128, 1024q] per pair

            with (
                tc.tile_pool(name="wp", bufs=1) as wp,
                tc.tile_pool(name="qkv", bufs=2) as qkvp,
                tc.tile_pool(name="vn", bufs=2) as vnp,
                tc.tile_pool(name="pe", bufs=10) as pep,
                tc.tile_pool(name="stg", bufs=1) as stgp,
                tc.tile_pool(name="rq", bufs=1) as rqp,
                tc.tile_pool(name="mm", bufs=2, space="PSUM") as mmp,
                tc.tile_pool(name="po", bufs=4, space="PSUM") as pop,
            ):
                for p in range(NP):
                    with nc.named_scope(f"pair{p}"):
                        # ---- weights for this head pair
                        w = []
                        for e in range(ET):
                            t = wp.tile([128, 384], F32R, name=f"w{e}")
                            nc.sync.dma_start(
                                out=t[:],
                                in_=wqkvT[e * 128:(e + 1) * 128,
                                          p * 384:(p + 1) * 384],
                            )
                            w.append(t)
                        bt = wp.tile([128, 3], F32, name="bias")
                        nc.sync.dma_start(out=bt[:], in_=bqkvP[p])

                        # ---- QKV projection (q only over query half = cols 0:1024)
                        qT2 = qkvp.tile([128, QTOK], F32R, name="qT2")
                        kT2 = qkvp.tile([128, S], F32R, name="kT2")
                        vT2 = qkvp.tile([128, S], F32R, name="vT2")
                        for ft, (dst, nch) in enumerate(
                            [(qT2, QC), (kT2, SC), (vT2, SC)]
                        ):
                            for cw in range(nch // 2):
                                mm = mmp.tile([128, 1024], F32, name="mm")
                                for half in range(2):
                                    ch = cw * 2 + half
                                    for e in range(ET):
                                        nc.tensor.matmul(
                                            mm[:, half * 512:(half + 1) * 512],
                                            w[e][:, ft * 128:(ft + 1) * 128],
                                            xt[e][:, ch * 512:(ch + 1) * 512],
                                            start=(e == 0), stop=(e == ET - 1),
                                        )
                                nc.vector.tensor_scalar_add(
                                    dst[:, cw * 1024:(cw + 1) * 1024],
                                    mm[:], bt[:, ft:ft + 1],
                                )

                        # ---- transpose v to natural layout + ones column
                        vns = [[None] * KT for _ in range(2)]
                        for h in range(2):
                            for kt in range(KT):
                                tp = mmp.tile([128, 1024], F32R, name="mm")
                                nc.tensor.transpose(
                                    tp[:, 0:64],
                                    vT2[h * 64:(h + 1) * 64,
                                        kt * 128:(kt + 1) * 128],
                                    id2[h * 64:(h + 1) * 64, :],
                                )
                                vn = vnp.tile([128, 65], BF16, name=f"vn{h}_{kt}")
                                nc.vector.tensor_copy(vn[:, 0:64], tp[:, 0:64])
                                nc.vector.tensor_copy(vn[:, 64:65], ones_t16[:])
                                vns[h][kt] = vn

                        # ---- attention
                        stage = stgp.tile([128, QTOK], F32, name="stage")
                        rqs = []
                        for h in range(2):
                            rq = rqp.tile([64, QTOK], F32, name=f"rq{h}")
                            nc.vector.memset(rq[:], 0.0)
                            rqs.append(rq)
                        groups = [(h, qc) for h in range(2) for qc in range(QC)]
                        pos = {}
                        for g in groups:
                            pos[g] = pop.tile([128, 512], F32, name="po")
                        KW = KT // 2
                        for (h, qc) in groups:
                            pes = [None] * KW
                            for kw in range(KW):
                                ps = mmp.tile([128, 1024], F32, name="mm")
                                for half in range(2):
                                    kt = kw * 2 + half
                                    nc.tensor.matmul(
                                        ps[:, half * 512:(half + 1) * 512],
                                        kT2[h * 64:(h + 1) * 64,
                                            kt * 128:(kt + 1) * 128],
                                        qT2[h * 64:(h + 1) * 64,
                                            qc * 512:(qc + 1) * 512],
                                        start=True, stop=True,
                                    )
                                pex = pep.tile([128, 1024], BF16, name="pe")
                                nc.scalar.activation(pex[:], ps[:], EXP)
                                pes[kw] = pex
                            po = pos[(h, qc)]
                            for kt in range(KT):
                                nc.tensor.matmul(
                                    po[:65, :], vns[h][kt][:],
                                    pes[kt // 2][:, (kt % 2) * 512:(kt % 2 + 1) * 512],
                                    start=(kt == 0), stop=(kt == KT - 1),
                                )
                        for (h, qc) in groups:
                            po = pos[(h, qc)]
                            nc.vector.tensor_copy(
                                stage[h * 64:(h + 1) * 64,
                                      qc * 512:(qc + 1) * 512],
                                po[0:64, :],
                            )
                            nc.vector.reciprocal(
                                rqs[h][0:1, qc * 512:(qc + 1) * 512],
                                po[64:65, :],
                            )

                        # ---- normalize (broadcast recip over 64 partitions via PE)
                        ou = outup.tile([128, QTOK], F32R, name=f"outU{p}")
                        for qc in range(QC):
                            bcc = pop.tile([128, 512], F32, name="po")
                            for h in range(2):
                                nc.tensor.matmul(
                                    bcc[h * 64:(h + 1) * 64, :],
                                    bsel_t[:],
                                    rqs[h][:, qc * 512:(qc + 1) * 512],
                                    start=True, stop=True,
                                )
                            nc.vector.tensor_mul(
                                ou[:, qc * 512:(qc + 1) * 512],
                                stage[:, qc * 512:(qc + 1) * 512],
                                bcc[:],
                            )
                        outur.append(ou)

            # ---- output projection
            with (
                tc.tile_pool(name="wo", bufs=1) as wop,
                tc.tile_pool(name="yp", bufs=3) as yp,
                tc.tile_pool(name="mm2", bufs=4, space="PSUM") as mm2p,
            ):
                wo = []
                for ft in range(ET):
                    t = wop.tile([128, E], F32R, name=f"wo{ft}")
                    nc.sync.dma_start(
                        out=t[:], in_=woutT[ft * 128:(ft + 1) * 128, :]
                    )
                    wo.append(t)
                bo = wop.tile([128, ET], F32, name="bo")
                nc.sync.dma_start(out=bo[:], in_=boutP[:])
                with nc.named_scope("outproj"):
                    for et in range(ET):
                        for qc in range(QC):
                            mm2 = mm2p.tile([128, 512], F32, name="mm2")
                            for ft in range(ET):
                                nc.tensor.matmul(
                                    mm2[:],
                                    wo[ft][:, et * 128:(et + 1) * 128],
                                    outur[ft][:, qc * 512:(qc + 1) * 512],
                                    start=(ft == 0), stop=(ft == ET - 1),
                                )
                            ysb = yp.tile([128, 512], F32, name="ysb")
                            nc.vector.tensor_scalar_add(
                                ysb[:], mm2[:], bo[:, et:et + 1]
                            )
                            nc.sync.dma_start(
                                out=yT[et * 128:(et + 1) * 128,
                                       qc * 512:(qc + 1) * 512],
                                in_=ysb[:],
                            )
    nc.finalize()
    return nc


def _host_inputs(x, Wqkv, bqkv, Wout, bout):
    x = np.asarray(x, dtype=np.float32)
    Wqkv = np.asarray(Wqkv, dtype=np.float32)
    bqkv = np.asarray(bqkv, dtype=np.float32)
    Wout = np.asarray(Wout, dtype=np.float32)
    bout = np.asarray(bout, dtype=np.float32)

    wqkvT = np.empty((E, 3 * E), np.float32)
    bqkvP = np.empty((NP, 128, 3), np.float32)
    for p in range(NP):
        r = slice(p * 128, (p + 1) * 128)
        wqkvT[:, p * 384:p * 384 + 128] = (Wqkv[r] * SCALE).T
        wqkvT[:, p * 384 + 128:p * 384 + 256] = Wqkv[E:][r].T
        wqkvT[:, p * 384 + 256:p * 384 + 384] = Wqkv[2 * E:][r].T
        bqkvP[p, :, 0] = bqkv[r] * SCALE
        bqkvP[p, :, 1] = bqkv[E:][r]
        bqkvP[p, :, 2] = bqkv[2 * E:][r]

    woutT = np.ascontiguousarray(Wout.T)
    boutP = np.ascontiguousarray(bout.reshape(ET, 128).T)
    eye = np.eye(64, dtype=np.float32)
    ident2 = np.concatenate([eye, eye], 0)
    onesC = np.ones((128, 1), np.float32)
    bsel = np.concatenate(
        [np.ones((1, 64), np.float32), np.zeros((63, 64), np.float32)], 0
    )

    in_maps = []
    for c in range(NCORES):
        xb = x[c // 2]
        if c % 2:
            xb = np.concatenate([xb[QTOK:], xb[:QTOK]], 0)
        in_maps.append({
            "xT": np.ascontiguousarray(xb.T),
            "wqkvT": wqkvT,
            "bqkvP": bqkvP,
            "woutT": woutT,
            "boutP": boutP,
            "ident2": ident2,
            "onesC": onesC,
            "bsel": bsel,
        })
    return in_maps


def kernel(x, Wqkv, bqkv, Wout, bout):
    global _NC, LAST_EXEC_NS, LAST_RESULTS
    from concourse.bass_utils import run_bass_kernel_spmd

    if _NC is None:
        _NC = build()
    in_maps = _host_inputs(x, Wqkv, bqkv, Wout, bout)
    trace = bool(os.environ.get("ATTN_TRACE"))
    res = run_bass_kernel_spmd(
        _NC, in_maps, core_ids=list(range(NCORES)), trace=trace
    )
    LAST_EXEC_NS = res.exec_time_ns
    LAST_RESULTS = res

    out = np.empty((B, S, E), np.float32)
    for c in range(NCORES):
        b, half = c // 2, c % 2
        out[b, half * QTOK:(half + 1) * QTOK, :] = res.results[c]["yT"].T
    return out
